# revision 8
# baseline (speedup 1.0000x reference)
"""Cost-sensitive loss (CE + cost-matrix lookup) on Trainium2, 8-core data-parallel.

Device work (per core, shard of 32768 rows x 1000 classes, fp32):
  - Stream x in [128, 1000] tiles (2 tiles per DMA).
  - DVE: one blockwise max reduce per tile ([128, 25, 40] -> [128, 25]).
  - ACT: exp(x) with accum_out -> per-row sum(exp) (no max-shift needed;
    |x| <= ~6 so exp never overflows fp32).
  - Exact argmax via hierarchy, batched 8 tiles at a time:
      per-tile max = strided reduce over the 8x25 group maxima,
      max_index over the 200 group maxima -> which 40-wide block per tile,
      per-tile indirect-DMA gather of the winning 40-elem block from HBM
      (HW indirect DMA semantics: one offset per partition, contiguous
      payload -- so one gather instruction per tile),
      one batched max_index over the 8 gathered blocks -> position within.
  - Outputs: per-partition partials [128,1] = sum_t log(sumexp) and the
    predicted argmax table preds [128, 256] (int32).

Host work (O(N) index arithmetic + table lookups):
  - x[row, label[row]] extraction, cost_matrix[label, pred] lookup,
    final sums / division by N.
"""

import numpy as np

import concourse.bacc as bacc
import concourse.bass as bass
import concourse.mybir as mybir
import concourse.tile as tile
from concourse import bass_utils

N = 262144
C = 1000
NCORES = 8
NS = N // NCORES          # 32768 rows per core
P = 128
GK = 40                   # candidate block width (elements)
NG = C // GK              # 25 blocks per row
TB = 8                    # tiles per argmax batch (max_index in_max width)
TPD = 4                   # tiles per streaming DMA

F32 = mybir.dt.float32
I32 = mybir.dt.int32
U32 = mybir.dt.uint32

_CACHE: dict = {}


def _body(tc, nc, x, pbase, tconst, partials, preds, nt):
    from contextlib import ExitStack

    nb = nt // TB
    ap_x = x.ap()                                               # [nrows*NG, GK]
    x_tiles = ap_x.rearrange("(t p g) k -> p t (g k)", t=nt, p=P, g=NG)
    AX = mybir.AxisListType.X
    ALU = mybir.AluOpType

    with ExitStack() as ctx:
        const = ctx.enter_context(tc.tile_pool(name="const", bufs=1))
        pbase_sb = const.tile([P, 1], I32)
        tconst_sb = const.tile([P, TB], I32)
        s_acc = const.tile([P, nt], F32)
        pr_acc = const.tile([P, nt], I32)
        esc = const.tile([P, C], F32)

        nc.sync.dma_start(out=pbase_sb[:], in_=pbase.ap())
        nc.sync.dma_start(out=tconst_sb[:], in_=tconst.ap())

        work = ctx.enter_context(tc.tile_pool(name="work", bufs=3))
        xp = ctx.enter_context(tc.tile_pool(name="xp", bufs=6))

        def finish_batch(st):
            """Tail of a batch's argmax: runs one batch late so the DVE
            never stalls on the POOL gather chain."""
            t0, m8, g8, gbuf = st
            pos8 = work.tile([P, TB], U32, tag="pos8")
            nc.vector.max_index(out=pos8[:], in_max=m8[:], in_values=gbuf[:])
            # pred = 40*g8 + pos8 - 1040*th   (tconst holds -1040*th)
            tmp8 = work.tile([P, TB], I32, tag="tmp8")
            nc.vector.scalar_tensor_tensor(
                out=tmp8[:], in0=g8[:], scalar=float(GK), in1=pos8[:],
                op0=ALU.mult, op1=ALU.add,
            )
            nc.vector.tensor_tensor(
                out=pr_acc[:, t0:t0 + TB], in0=tmp8[:], in1=tconst_sb[:],
                op=ALU.add,
            )

        pending = None
        for b in range(nb):
            t0 = b * TB
            gm = work.tile([P, TB * NG], F32, tag="gm")
            xts = []
            for j in range(TB // TPD):
                xt = xp.tile([P, TPD * C], F32, tag="xt")
                nc.sync.dma_start(
                    out=xt[:].rearrange("p (j c) -> p j c", c=C),
                    in_=x_tiles[:, t0 + j * TPD: t0 + (j + 1) * TPD, :],
                )
                xts.append(xt)
            for th in range(TB):
                sl = xts[th // TPD][:, (th % TPD) * C:(th % TPD + 1) * C]
                nc.vector.reduce_max(
                    out=gm[:, th * NG:(th + 1) * NG],
                    in_=sl.rearrange("p (g k) -> p g k", k=GK),
                    axis=AX,
                )
                nc.scalar.activation(
                    out=esc[:],
                    in_=sl,
                    func=mybir.ActivationFunctionType.Exp,
                    accum_out=s_acc[:, t0 + th: t0 + th + 1],
                )
            # Per-tile maxima of this batch of 8 tiles.
            m8 = work.tile([P, TB], F32, tag="m8")
            nc.vector.reduce_max(
                out=m8[:], in_=gm[:].rearrange("p (t g) -> p t g", g=NG), axis=AX
            )
            g8 = work.tile([P, TB], U32, tag="g8")
            nc.vector.max_index(out=g8[:], in_max=m8[:], in_values=gm[:])
            # Gather each tile's winning 40-elem block: one [128,1]-offset
            # indirect DMA per tile (HW: one descriptor per partition).
            gbuf = work.tile([P, TB * GK], F32, tag="gbuf")
            for th in range(TB):
                t = t0 + th
                goff = work.tile([P, 1], I32, tag=f"goff{th}")
                # block row-index = g8 + 25p + (3200*t - 25*th)
                nc.vector.scalar_tensor_tensor(
                    out=goff[:], in0=g8[:, th:th + 1],
                    scalar=float(NG * P * t - NG * th),
                    in1=pbase_sb[:], op0=ALU.add, op1=ALU.add,
                )
                nc.gpsimd.indirect_dma_start(
                    out=gbuf[:, th * GK:(th + 1) * GK],
                    out_offset=None,
                    in_=ap_x,
                    in_offset=bass.IndirectOffsetOnAxis(ap=goff[:], axis=0),
                    bounds_check=nt * P * NG - 1,
                    oob_is_err=False,
                )
            if pending is not None:
                finish_batch(pending)
            pending = (t0, m8, g8, gbuf)
        finish_batch(pending)

        # Epilogue: per-partition sum of log(sumexp).
        ls = const.tile([P, nt], F32)
        nc.scalar.activation(
            out=ls[:], in_=s_acc[:], func=mybir.ActivationFunctionType.Ln
        )
        p1 = const.tile([P, 1], F32)
        nc.vector.reduce_sum(out=p1[:], in_=ls[:], axis=AX)
        nc.sync.dma_start(out=partials.ap(), in_=p1[:])
        nc.sync.dma_start(out=preds.ap(), in_=pr_acc[:])


def build_module(nt=NS // P):
    nc = bacc.Bacc(
        "TRN2",
        target_bir_lowering=False,
        debug=False,
        enable_asserts=False,
        num_devices=NCORES,
    )
    x = nc.dram_tensor("x", [nt * P * NG, GK], F32, kind="ExternalInput")
    pbase = nc.dram_tensor("pbase", [P, 1], I32, kind="ExternalInput")
    tconst = nc.dram_tensor("tconst", [P, TB], I32, kind="ExternalInput")
    partials = nc.dram_tensor("partials", [P, 1], F32, kind="ExternalOutput")
    preds = nc.dram_tensor("preds", [P, nt], I32, kind="ExternalOutput")
    with tile.TileContext(nc) as tc:
        _body(tc, nc, x, pbase, tconst, partials, preds, nt)
    nc.compile()
    return nc


def host_inputs(nt=NS // P, ncores=NCORES, x=None):
    """Per-core input maps. x is the full [N, C] fp32 array."""
    ns = nt * P
    pb = (NG * np.arange(P, dtype=np.int64)[:, None]).astype(np.int32)
    tc_ = (-(C + GK) * np.arange(TB, dtype=np.int64)[None, :]
           ).astype(np.int32) * np.ones((P, 1), dtype=np.int32)
    in_maps = []
    for cidx in range(ncores):
        in_maps.append({
            "x": x[cidx * ns:(cidx + 1) * ns].reshape(ns * NG, GK),
            "pbase": pb,
            "tconst": tc_,
        })
    return in_maps


def combine(results, x, lab, cost_matrix, nt=NS // P):
    """Host-side finish: ce = sum(log sumexp) - sum(x[label]); cost lookup."""
    ns = nt * P
    n_total = len(results) * ns
    lse_sum = 0.0
    preds_all = []
    for r in results:
        lse_sum += np.asarray(r["partials"], dtype=np.float64).sum()
        # preds[p, t] is the argmax of shard row 128*t + p
        preds_all.append(np.asarray(r["preds"]).T.reshape(-1))
    preds = np.concatenate(preds_all)
    preds = np.clip(preds, 0, C - 1)
    xlab_sum = np.take_along_axis(
        x, lab[: len(preds), None].astype(np.int64), axis=1
    )[:, 0].astype(np.float64).sum()
    cost_sum = np.asarray(cost_matrix)[
        lab[: len(preds)].astype(np.int64), preds
    ].astype(np.float64).sum()
    ce = (lse_sum - xlab_sum) / n_total
    cost = cost_sum / n_total
    return np.float32(ce + cost)


def kernel(outputs, labels, cost_matrix):
    if "nc" not in _CACHE:
        _CACHE["nc"] = build_module()
    nc = _CACHE["nc"]
    x = np.ascontiguousarray(np.asarray(outputs), dtype=np.float32)
    lab = np.asarray(labels)
    in_maps = host_inputs(x=x)
    res = bass_utils.run_bass_kernel_spmd(nc, in_maps, core_ids=list(range(NCORES)))
    return combine(res.results, x, lab, cost_matrix)


# revision 9
# speedup vs baseline: 1.0791x; 1.0791x over previous
"""Cost-sensitive loss (CE + cost-matrix lookup) on Trainium2, 8-core data-parallel.

Device work (per core, shard of 32768 rows x 1000 classes, fp32):
  - Stream x in [128, 1000] tiles (2 tiles per DMA).
  - DVE: one blockwise max reduce per tile ([128, 25, 40] -> [128, 25]).
  - ACT: exp(x) with accum_out -> per-row sum(exp) (no max-shift needed;
    |x| <= ~6 so exp never overflows fp32).
  - Exact argmax via hierarchy, batched 8 tiles at a time:
      per-tile max = strided reduce over the 8x25 group maxima,
      max_index over the 200 group maxima -> which 40-wide block per tile,
      per-tile indirect-DMA gather of the winning 40-elem block from HBM
      (HW indirect DMA semantics: one offset per partition, contiguous
      payload -- so one gather instruction per tile),
      one batched max_index over the 8 gathered blocks -> position within.
  - Outputs: per-partition partials [128,1] = sum_t log(sumexp) and the
    predicted argmax table preds [128, 256] (int32).

Host work (O(N) index arithmetic + table lookups):
  - x[row, label[row]] extraction, cost_matrix[label, pred] lookup,
    final sums / division by N.
"""

import numpy as np

import concourse.bacc as bacc
import concourse.bass as bass
import concourse.mybir as mybir
import concourse.tile as tile
from concourse import bass_utils

N = 262144
C = 1000
NCORES = 8
NS = N // NCORES          # 32768 rows per core
P = 128
GK = 40                   # candidate block width (elements)
NG = C // GK              # 25 blocks per row
TB = 8                    # tiles per argmax batch (max_index in_max width)
TPD = 2                   # tiles per streaming DMA

F32 = mybir.dt.float32
I32 = mybir.dt.int32
U32 = mybir.dt.uint32

_CACHE: dict = {}


def _body(tc, nc, x, pbase, tconst, partials, preds, nt):
    from contextlib import ExitStack

    nb = nt // TB
    ap_x = x.ap()                                               # [nrows*NG, GK]
    x_tiles = ap_x.rearrange("(t p g) k -> p t (g k)", t=nt, p=P, g=NG)
    AX = mybir.AxisListType.X
    ALU = mybir.AluOpType

    with ExitStack() as ctx:
        const = ctx.enter_context(tc.tile_pool(name="const", bufs=1))
        pbase_sb = const.tile([P, 1], I32)
        tconst_sb = const.tile([P, TB], I32)
        s_acc = const.tile([P, nt], F32)
        pr_acc = const.tile([P, nt], I32)
        esc = const.tile([P, C], F32)

        nc.sync.dma_start(out=pbase_sb[:], in_=pbase.ap())
        nc.sync.dma_start(out=tconst_sb[:], in_=tconst.ap())

        work = ctx.enter_context(tc.tile_pool(name="work", bufs=3))
        xp = ctx.enter_context(tc.tile_pool(name="xp", bufs=12))

        def finish_batch(st):
            """Tail of a batch's argmax: runs one batch late so the DVE
            never stalls on the POOL gather chain."""
            t0, m8, g8, gbuf = st
            pos8 = work.tile([P, TB], U32, tag="pos8")
            nc.vector.max_index(out=pos8[:], in_max=m8[:], in_values=gbuf[:])
            # pred = 40*g8 + pos8 - 1040*th   (tconst holds -1040*th)
            tmp8 = work.tile([P, TB], I32, tag="tmp8")
            nc.vector.scalar_tensor_tensor(
                out=tmp8[:], in0=g8[:], scalar=float(GK), in1=pos8[:],
                op0=ALU.mult, op1=ALU.add,
            )
            nc.vector.tensor_tensor(
                out=pr_acc[:, t0:t0 + TB], in0=tmp8[:], in1=tconst_sb[:],
                op=ALU.add,
            )

        pending = []
        for b in range(nb):
            t0 = b * TB
            gm = work.tile([P, TB * NG], F32, tag="gm")
            xts = []
            for j in range(TB // TPD):
                xt = xp.tile([P, TPD * C], F32, tag="xt")
                nc.sync.dma_start(
                    out=xt[:].rearrange("p (j c) -> p j c", c=C),
                    in_=x_tiles[:, t0 + j * TPD: t0 + (j + 1) * TPD, :],
                )
                xts.append(xt)
            for th in range(TB):
                sl = xts[th // TPD][:, (th % TPD) * C:(th % TPD + 1) * C]
                nc.vector.reduce_max(
                    out=gm[:, th * NG:(th + 1) * NG],
                    in_=sl.rearrange("p (g k) -> p g k", k=GK),
                    axis=AX,
                )
                nc.scalar.activation(
                    out=esc[:],
                    in_=sl,
                    func=mybir.ActivationFunctionType.Exp,
                    accum_out=s_acc[:, t0 + th: t0 + th + 1],
                )
            # Per-tile maxima of this batch of 8 tiles.
            m8 = work.tile([P, TB], F32, tag="m8")
            nc.vector.reduce_max(
                out=m8[:], in_=gm[:].rearrange("p (t g) -> p t g", g=NG), axis=AX
            )
            g8 = work.tile([P, TB], U32, tag="g8")
            nc.vector.max_index(out=g8[:], in_max=m8[:], in_values=gm[:])
            # Gather each tile's winning 40-elem block: one [128,1]-offset
            # indirect DMA per tile (HW: one descriptor per partition).
            gbuf = work.tile([P, TB * GK], F32, tag="gbuf")
            for th in range(TB):
                t = t0 + th
                goff = work.tile([P, 1], I32, tag=f"goff{th}")
                # block row-index = g8 + 25p + (3200*t - 25*th)
                nc.vector.scalar_tensor_tensor(
                    out=goff[:], in0=g8[:, th:th + 1],
                    scalar=float(NG * P * t - NG * th),
                    in1=pbase_sb[:], op0=ALU.add, op1=ALU.add,
                )
                nc.gpsimd.indirect_dma_start(
                    out=gbuf[:, th * GK:(th + 1) * GK],
                    out_offset=None,
                    in_=ap_x,
                    in_offset=bass.IndirectOffsetOnAxis(ap=goff[:], axis=0),
                    bounds_check=nt * P * NG - 1,
                    oob_is_err=False,
                )
            pending.append((t0, m8, g8, gbuf))
            if len(pending) > 2:
                finish_batch(pending.pop(0))
        for st in pending:
            finish_batch(st)

        # Epilogue: per-partition sum of log(sumexp).
        ls = const.tile([P, nt], F32)
        nc.scalar.activation(
            out=ls[:], in_=s_acc[:], func=mybir.ActivationFunctionType.Ln
        )
        p1 = const.tile([P, 1], F32)
        nc.vector.reduce_sum(out=p1[:], in_=ls[:], axis=AX)
        nc.sync.dma_start(out=partials.ap(), in_=p1[:])
        nc.sync.dma_start(out=preds.ap(), in_=pr_acc[:])


def build_module(nt=NS // P):
    nc = bacc.Bacc(
        "TRN2",
        target_bir_lowering=False,
        debug=False,
        enable_asserts=False,
        num_devices=NCORES,
    )
    x = nc.dram_tensor("x", [nt * P * NG, GK], F32, kind="ExternalInput")
    pbase = nc.dram_tensor("pbase", [P, 1], I32, kind="ExternalInput")
    tconst = nc.dram_tensor("tconst", [P, TB], I32, kind="ExternalInput")
    partials = nc.dram_tensor("partials", [P, 1], F32, kind="ExternalOutput")
    preds = nc.dram_tensor("preds", [P, nt], I32, kind="ExternalOutput")
    with tile.TileContext(nc) as tc:
        _body(tc, nc, x, pbase, tconst, partials, preds, nt)
    nc.compile()
    return nc


def host_inputs(nt=NS // P, ncores=NCORES, x=None):
    """Per-core input maps. x is the full [N, C] fp32 array."""
    ns = nt * P
    pb = (NG * np.arange(P, dtype=np.int64)[:, None]).astype(np.int32)
    tc_ = (-(C + GK) * np.arange(TB, dtype=np.int64)[None, :]
           ).astype(np.int32) * np.ones((P, 1), dtype=np.int32)
    in_maps = []
    for cidx in range(ncores):
        in_maps.append({
            "x": x[cidx * ns:(cidx + 1) * ns].reshape(ns * NG, GK),
            "pbase": pb,
            "tconst": tc_,
        })
    return in_maps


def combine(results, x, lab, cost_matrix, nt=NS // P):
    """Host-side finish: ce = sum(log sumexp) - sum(x[label]); cost lookup."""
    ns = nt * P
    n_total = len(results) * ns
    lse_sum = 0.0
    preds_all = []
    for r in results:
        lse_sum += np.asarray(r["partials"], dtype=np.float64).sum()
        # preds[p, t] is the argmax of shard row 128*t + p
        preds_all.append(np.asarray(r["preds"]).T.reshape(-1))
    preds = np.concatenate(preds_all)
    preds = np.clip(preds, 0, C - 1)
    xlab_sum = np.take_along_axis(
        x, lab[: len(preds), None].astype(np.int64), axis=1
    )[:, 0].astype(np.float64).sum()
    cost_sum = np.asarray(cost_matrix)[
        lab[: len(preds)].astype(np.int64), preds
    ].astype(np.float64).sum()
    ce = (lse_sum - xlab_sum) / n_total
    cost = cost_sum / n_total
    return np.float32(ce + cost)


def kernel(outputs, labels, cost_matrix):
    if "nc" not in _CACHE:
        _CACHE["nc"] = build_module()
    nc = _CACHE["nc"]
    x = np.ascontiguousarray(np.asarray(outputs), dtype=np.float32)
    lab = np.asarray(labels)
    in_maps = host_inputs(x=x)
    res = bass_utils.run_bass_kernel_spmd(nc, in_maps, core_ids=list(range(NCORES)))
    return combine(res.results, x, lab, cost_matrix)


# revision 10
# speedup vs baseline: 1.0901x; 1.0102x over previous
"""Cost-sensitive loss (CE + cost-matrix lookup) on Trainium2, 8-core data-parallel.

Device work (per core, shard of 32768 rows x 1000 classes, fp32):
  - Stream x in [128, 1000] tiles (2 tiles per DMA).
  - DVE: one blockwise max reduce per tile ([128, 25, 40] -> [128, 25]).
  - ACT: exp(x) with accum_out -> per-row sum(exp) (no max-shift needed;
    |x| <= ~6 so exp never overflows fp32).
  - Exact argmax via hierarchy, batched 8 tiles at a time:
      per-tile max = strided reduce over the 8x25 group maxima,
      max_index over the 200 group maxima -> which 40-wide block per tile,
      per-tile indirect-DMA gather of the winning 40-elem block from HBM
      (HW indirect DMA semantics: one offset per partition, contiguous
      payload -- so one gather instruction per tile),
      one batched max_index over the 8 gathered blocks -> position within.
  - Outputs: per-partition partials [128,1] = sum_t log(sumexp) and the
    predicted argmax table preds [128, 256] (int32).

Host work (O(N) index arithmetic + table lookups):
  - x[row, label[row]] extraction, cost_matrix[label, pred] lookup,
    final sums / division by N.
"""

import numpy as np

import concourse.bacc as bacc
import concourse.bass as bass
import concourse.mybir as mybir
import concourse.tile as tile
from concourse import bass_utils

N = 262144
C = 1000
NCORES = 8
NS = N // NCORES          # 32768 rows per core
P = 128
GK = 40                   # candidate block width (elements)
NG = C // GK              # 25 blocks per row
TB = 8                    # tiles per argmax batch (max_index in_max width)
TPD = 2                   # tiles per streaming DMA

F32 = mybir.dt.float32
I32 = mybir.dt.int32
U32 = mybir.dt.uint32

_CACHE: dict = {}


def _body(tc, nc, x, pbase, tconst, partials, preds, nt):
    from contextlib import ExitStack

    nb = nt // TB
    ap_x = x.ap()                                               # [nrows*NG, GK]
    x_tiles = ap_x.rearrange("(t p g) k -> p t (g k)", t=nt, p=P, g=NG)
    AX = mybir.AxisListType.X
    ALU = mybir.AluOpType

    with ExitStack() as ctx:
        const = ctx.enter_context(tc.tile_pool(name="const", bufs=1))
        pbase_sb = const.tile([P, 1], I32)
        tconst_sb = const.tile([P, TB], I32)
        s_acc = const.tile([P, nt], F32)
        pr_acc = const.tile([P, nt], I32)
        esc = const.tile([P, C], F32)

        nc.sync.dma_start(out=pbase_sb[:], in_=pbase.ap())
        nc.sync.dma_start(out=tconst_sb[:], in_=tconst.ap())

        work = ctx.enter_context(tc.tile_pool(name="work", bufs=6))
        xp = ctx.enter_context(tc.tile_pool(name="xp", bufs=12))

        def finish_batch(st):
            """Tail of a batch's argmax: runs one batch late so the DVE
            never stalls on the POOL gather chain."""
            t0, m8, g8, gbuf = st
            pos8 = work.tile([P, TB], U32, tag="pos8")
            nc.vector.max_index(out=pos8[:], in_max=m8[:], in_values=gbuf[:])
            # pred = 40*g8 + pos8 - 1040*th   (tconst holds -1040*th)
            tmp8 = work.tile([P, TB], I32, tag="tmp8")
            nc.vector.scalar_tensor_tensor(
                out=tmp8[:], in0=g8[:], scalar=float(GK), in1=pos8[:],
                op0=ALU.mult, op1=ALU.add,
            )
            nc.vector.tensor_tensor(
                out=pr_acc[:, t0:t0 + TB], in0=tmp8[:], in1=tconst_sb[:],
                op=ALU.add,
            )

        pending = []
        for b in range(nb):
            t0 = b * TB
            gm = work.tile([P, TB * NG], F32, tag="gm")
            xts = []
            for j in range(TB // TPD):
                xt = xp.tile([P, TPD * C], F32, tag="xt")
                nc.sync.dma_start(
                    out=xt[:].rearrange("p (j c) -> p j c", c=C),
                    in_=x_tiles[:, t0 + j * TPD: t0 + (j + 1) * TPD, :],
                )
                xts.append(xt)
            for th in range(TB):
                sl = xts[th // TPD][:, (th % TPD) * C:(th % TPD + 1) * C]
                nc.vector.reduce_max(
                    out=gm[:, th * NG:(th + 1) * NG],
                    in_=sl.rearrange("p (g k) -> p g k", k=GK),
                    axis=AX,
                )
                nc.scalar.activation(
                    out=esc[:],
                    in_=sl,
                    func=mybir.ActivationFunctionType.Exp,
                    accum_out=s_acc[:, t0 + th: t0 + th + 1],
                )
            # Per-tile maxima of this batch of 8 tiles.
            m8 = work.tile([P, TB], F32, tag="m8")
            nc.vector.reduce_max(
                out=m8[:], in_=gm[:].rearrange("p (t g) -> p t g", g=NG), axis=AX
            )
            g8 = work.tile([P, TB], U32, tag="g8")
            nc.vector.max_index(out=g8[:], in_max=m8[:], in_values=gm[:])
            # Gather each tile's winning 40-elem block: one [128,1]-offset
            # indirect DMA per tile (HW: one descriptor per partition).
            gbuf = work.tile([P, TB * GK], F32, tag="gbuf")
            for th in range(TB):
                t = t0 + th
                goff = work.tile([P, 1], I32, tag=f"goff{th}")
                # block row-index = g8 + 25p + (3200*t - 25*th)
                nc.vector.scalar_tensor_tensor(
                    out=goff[:], in0=g8[:, th:th + 1],
                    scalar=float(NG * P * t - NG * th),
                    in1=pbase_sb[:], op0=ALU.add, op1=ALU.add,
                )
                nc.gpsimd.indirect_dma_start(
                    out=gbuf[:, th * GK:(th + 1) * GK],
                    out_offset=None,
                    in_=ap_x,
                    in_offset=bass.IndirectOffsetOnAxis(ap=goff[:], axis=0),
                    bounds_check=nt * P * NG - 1,
                    oob_is_err=False,
                )
            pending.append((t0, m8, g8, gbuf))
            if len(pending) > 4:
                finish_batch(pending.pop(0))
        for st in pending:
            finish_batch(st)

        # Epilogue: per-partition sum of log(sumexp).
        ls = const.tile([P, nt], F32)
        nc.scalar.activation(
            out=ls[:], in_=s_acc[:], func=mybir.ActivationFunctionType.Ln
        )
        p1 = const.tile([P, 1], F32)
        nc.vector.reduce_sum(out=p1[:], in_=ls[:], axis=AX)
        nc.sync.dma_start(out=partials.ap(), in_=p1[:])
        nc.sync.dma_start(out=preds.ap(), in_=pr_acc[:])


def build_module(nt=NS // P):
    nc = bacc.Bacc(
        "TRN2",
        target_bir_lowering=False,
        debug=False,
        enable_asserts=False,
        num_devices=NCORES,
    )
    x = nc.dram_tensor("x", [nt * P * NG, GK], F32, kind="ExternalInput")
    pbase = nc.dram_tensor("pbase", [P, 1], I32, kind="ExternalInput")
    tconst = nc.dram_tensor("tconst", [P, TB], I32, kind="ExternalInput")
    partials = nc.dram_tensor("partials", [P, 1], F32, kind="ExternalOutput")
    preds = nc.dram_tensor("preds", [P, nt], I32, kind="ExternalOutput")
    with tile.TileContext(nc) as tc:
        _body(tc, nc, x, pbase, tconst, partials, preds, nt)
    nc.compile()
    return nc


def host_inputs(nt=NS // P, ncores=NCORES, x=None):
    """Per-core input maps. x is the full [N, C] fp32 array."""
    ns = nt * P
    pb = (NG * np.arange(P, dtype=np.int64)[:, None]).astype(np.int32)
    tc_ = (-(C + GK) * np.arange(TB, dtype=np.int64)[None, :]
           ).astype(np.int32) * np.ones((P, 1), dtype=np.int32)
    in_maps = []
    for cidx in range(ncores):
        in_maps.append({
            "x": x[cidx * ns:(cidx + 1) * ns].reshape(ns * NG, GK),
            "pbase": pb,
            "tconst": tc_,
        })
    return in_maps


def combine(results, x, lab, cost_matrix, nt=NS // P):
    """Host-side finish: ce = sum(log sumexp) - sum(x[label]); cost lookup."""
    ns = nt * P
    n_total = len(results) * ns
    lse_sum = 0.0
    preds_all = []
    for r in results:
        lse_sum += np.asarray(r["partials"], dtype=np.float64).sum()
        # preds[p, t] is the argmax of shard row 128*t + p
        preds_all.append(np.asarray(r["preds"]).T.reshape(-1))
    preds = np.concatenate(preds_all)
    preds = np.clip(preds, 0, C - 1)
    xlab_sum = np.take_along_axis(
        x, lab[: len(preds), None].astype(np.int64), axis=1
    )[:, 0].astype(np.float64).sum()
    cost_sum = np.asarray(cost_matrix)[
        lab[: len(preds)].astype(np.int64), preds
    ].astype(np.float64).sum()
    ce = (lse_sum - xlab_sum) / n_total
    cost = cost_sum / n_total
    return np.float32(ce + cost)


def kernel(outputs, labels, cost_matrix):
    if "nc" not in _CACHE:
        _CACHE["nc"] = build_module()
    nc = _CACHE["nc"]
    x = np.ascontiguousarray(np.asarray(outputs), dtype=np.float32)
    lab = np.asarray(labels)
    in_maps = host_inputs(x=x)
    res = bass_utils.run_bass_kernel_spmd(nc, in_maps, core_ids=list(range(NCORES)))
    return combine(res.results, x, lab, cost_matrix)
